# revision 23
# baseline (speedup 1.0000x reference)
"""Trainium2 Bass kernel for nn_Blocks_86096914416144.

Spiking-neuron block scan: T=1024 steps in 128 sequential blocks of tb=8,
B=32, N=1024, fp32. Sharding: channel dim N split 8 ways across cores
(pure data parallel; per-channel params private per core).

Per-core layout: SBUF tiles [128 partitions = channel n, free = (b:32,
slot:9)] with slot innermost (slot 0 = carry injection, slots 1..8 = tau).
All within-block recurrences are DVE tensor_tensor_scan ops over the
288-wide free dim; segment resets come from 0-entries in the const
pattern tiles at slot 0.

v6 (restructured from v5 after trace analysis): the baseline spent ~7.1us
of each ~8.5us block period in two pathological ops (a Pool TENSOR_SCALAR
at 4.0us and a Vector STT whose broadcast operand lowered to an
inner-stride-8 AP at 3.1us). v6 keeps the whole serial chain on the
Vector engine with dense/contiguous APs only, moves the first-spike
extraction to Pool, and shortens the chain:

  f9 = mh9 > a           (TT; mh9/f9 slot0 pinned so f9[0]=0)
  e9 = maxscan(seg,f9)   (segmented any-spike-yet flag; e9[0]=0)
  h9 = -8*e9 + e9[8]     (STT; slot0 gives h=any, so one is_lt covers both
                          the refractory mask AND the v_init gate)
  d29 = (h9<1)*vx9       (vx9 = [m_last | x] staged in PSUM by PE/Act)
  m9 = scan(bpat9,d29)   (membrane; bpat9 slot0=0 resets state)
  mh9 = (m9-1)*invg      (threshold-normalized membrane, slots 1..8)
  w  = e9[:8] + e9[1:]   (POOL TT; w==1 <=> first spike at that step)
  spk = (w==1)*q         (STT; q = p^(6-tau), one-hot at first spike)
  u  = reduce_add(spk)   ([32]; = p^(6-tau*) if spiked else 0)
  a' = a*p8 + u          (adaptation carry, closed form)
  out block = spk  (host: y = (spk > 0))

Scheduling notes: the tile scheduler enqueues per-engine by sim-ready
time with an optimistic Pool cost model, so spk/u/a' carry a sim-only
tile_wait_until floor to keep them behind the m9 chain; only ONE Pool op
runs per block because a second overlaps the m9 scan and SBUF-port
contention slows that scan ~60%.

All arithmetic is bit-exact vs the validated v5 formulation (integer
count algebra in fp32; same m/mh/a expressions).
"""

import os
import sys

import numpy as np

T_LEN = 1024
TB = 8
TBP = TB + 1  # 9 slots: slot 0 carries v_init into the scan
B = 32
N = 1024
NCORES = 8
NSH = N // NCORES  # 128 channels per core
BH = 2
BL = 16  # b = bh*16 + bl
FD = B * TB  # 256
FD9 = B * TBP  # 288

_MODULE_CACHE = {}


def _import_concourse():
    try:
        import concourse  # noqa: F401
    except ImportError:
        for p in ("/opt/trn_rl_repo", "/root/.axon_site/_ro/trn_rl_repo"):
            if os.path.isdir(p) and p not in sys.path:
                sys.path.insert(0, p)
        import concourse  # noqa: F401


def _build_module(t_len, repeats=1, rep_reset=True, variant="full"):
    """Build + compile the per-core Bass module (SPMD: same NEFF, 8 cores)."""
    _import_concourse()
    from contextlib import ExitStack, nullcontext

    import concourse.bacc as bacc
    import concourse.bass as bass
    import concourse.tile as tile
    from concourse import masks, mybir

    nblk = t_len // TB
    f32 = mybir.dt.float32
    bf16 = mybir.dt.bfloat16
    Alu = mybir.AluOpType
    AF = mybir.ActivationFunctionType
    Ax = mybir.AxisListType

    nc = bacc.Bacc("TRN2", target_bir_lowering=False, debug=False)

    x_d = nc.dram_tensor("x", [t_len, B, NSH], f32, kind="ExternalInput")
    bpat_d = nc.dram_tensor("bpat", [128, FD9], f32, kind="ExternalInput")
    segpat_d = nc.dram_tensor("segpat", [128, FD9], f32, kind="ExternalInput")
    qt_d = nc.dram_tensor("qt", [128, FD], f32, kind="ExternalInput")
    invg_d = nc.dram_tensor("invg", [128, FD], f32, kind="ExternalInput")
    prm_d = nc.dram_tensor("prm", [128, 1], f32, kind="ExternalInput")  # p^8
    y_d = nc.dram_tensor("y", [t_len, B, NSH], bf16, kind="ExternalOutput")

    def r3(ap, t=TB):  # [128, 32*t] -> [128, 32, t]
        return ap.rearrange("p (b t) -> p b t", t=t)

    def dram_block_ap(tens_ap, k, esz):
        # [(bl, tau) partition-order, (bh, n) free-order] view of block k of a
        # [t_len, B, NSH] dram tensor; element order matches a [128, 256] tile.
        return bass.AP(
            tensor=tens_ap.tensor,
            offset=k * TB * B * NSH,
            ap=[[NSH, BL], [B * NSH, TB], [BL * NSH, BH], [1, NSH]],
        )

    with tile.TileContext(nc) as tc, ExitStack() as ctx:
        const = ctx.enter_context(tc.tile_pool(name="const", bufs=1))
        state = ctx.enter_context(tc.tile_pool(name="state", bufs=1))
        xp = ctx.enter_context(tc.tile_pool(name="xp", bufs=4))
        outp = ctx.enter_context(tc.tile_pool(name="outp", bufs=4))
        mp = ctx.enter_context(tc.tile_pool(name="mp", bufs=3))
        cp = ctx.enter_context(tc.tile_pool(name="cp", bufs=3))
        dp = ctx.enter_context(tc.tile_pool(name="dp", bufs=3))
        hp = ctx.enter_context(tc.tile_pool(name="hp", bufs=3))
        fp = ctx.enter_context(tc.tile_pool(name="fp", bufs=2))
        wp = ctx.enter_context(tc.tile_pool(name="wp", bufs=2))
        spkp = ctx.enter_context(tc.tile_pool(name="spkp", bufs=2))
        up = ctx.enter_context(tc.tile_pool(name="up", bufs=2))
        ttp = ctx.enter_context(tc.tile_pool(name="ttp", bufs=2))
        psin = ctx.enter_context(tc.tile_pool(name="psin", bufs=3, space="PSUM"))
        psout = ctx.enter_context(tc.tile_pool(name="psout", bufs=4, space="PSUM"))

        # constants
        bpat = const.tile([128, FD9], f32)
        segpat = const.tile([128, FD9], f32)
        qt = const.tile([128, FD], f32)
        invg = const.tile([128, FD], f32)
        prm = const.tile([128, 1], f32)
        ident = const.tile([128, 128], f32)
        # const loads ride the ACT/Pool DMA queues so the x0/x1 block DMAs
        # (SP queue, critical path to block 0) are not queued behind them
        nc.scalar.dma_start(out=bpat[:], in_=bpat_d[:])
        nc.scalar.dma_start(out=segpat[:], in_=segpat_d[:])
        nc.scalar.dma_start(out=qt[:], in_=qt_d[:])
        masks.make_identity(nc, ident[:])
        nc.gpsimd.dma_start(out=invg[:], in_=invg_d[:])
        nc.gpsimd.dma_start(out=prm[:], in_=prm_d[:])
        p8_col = prm[:, 0:1]

        # persistent per-(n,b) state
        a0 = state.tile([128, B], f32)
        a1 = state.tile([128, B], f32)
        mh9 = state.tile([128, FD9], f32)  # slot0 pinned to -1
        mh_sl = r3(mh9[:], TBP)[:, :, 1:TBP]  # [128, 32, 8] strided view
        f9 = state.tile([128, FD9], f32)  # slot0 pinned to 0
        f_sl = r3(f9[:], TBP)[:, :, 1:TBP]

        def stage_x(k):
            # DMA block k, PE-transpose directly into vx9[k] slots 1..8 (PSUM)
            xN = xp.tile([128, FD], f32, tag="xN")
            nc.sync.dma_start(out=xN[:], in_=dram_block_ap(x_d[:], k, 4))
            vx = psin.tile([128, FD9], f32, tag="vx")
            vxv = r3(vx[:], TBP)
            nc.tensor.transpose(
                vxv[:, 0:BL, 1:TBP], xN[:, 0:128], ident[:]
            )
            nc.tensor.transpose(
                vxv[:, BL : 2 * BL, 1:TBP], xN[:, 128:256], ident[:]
            )
            return vx

        loop_cm = tc.For_i(0, repeats, 1) if repeats > 1 else nullcontext()
        with loop_cm:
            nc.vector.memset(a0[:], 0.0)
            nc.vector.memset(r3(mh9[:], TBP)[:, :, 0:1], -1.0)
            nc.vector.memset(r3(f9[:], TBP)[:, :, 0:1], 0.0)

            # ---------- prologue: x0/x1 prefetch, m9_0 / mh9_0 ----------
            # block 0 has no refractory history and v_init = 0: scan x directly
            vx_c = stage_x(0)
            nc.vector.memset(r3(vx_c[:], TBP)[:, :, 0:1], 0.0)
            vx_n = stage_x(1)

            m9 = mp.tile([128, FD9], f32, tag="m9")
            nc.vector.tensor_tensor_scan(
                out=m9[:], data0=bpat[:], data1=vx_c[:], initial=0.0,
                op0=Alu.mult, op1=Alu.add,
            )
            nc.vector.scalar_tensor_tensor(
                out=mh_sl, in0=r3(m9[:], TBP)[:, :, 1:TBP],
                scalar=1.0, in1=r3(invg[:]), op0=Alu.subtract, op1=Alu.mult,
            )
            # vx for block 1: slot0 = m_last of block 0 (gated by h9 later)
            nc.scalar.copy(
                out=r3(vx_n[:], TBP)[:, :, 0:1],
                in_=r3(m9[:], TBP)[:, :, 8:9],
            )

            a_in, a_out = a0, a1
            for k in range(nblk):
                last = k + 1 == nblk

                # ---- DVE chain ----
                # f9 = mh9 > a  (persistent tile; slot0 pre-zeroed once,
                # the TT writes only slots 1..8)
                nc.vector.tensor_tensor(
                    out=f_sl, in0=mh_sl,
                    in1=a_in[:].unsqueeze(2).broadcast_to([128, B, TB]),
                    op=Alu.is_gt,
                )
                # e9 = any-spike-yet (segmented max-scan; slot0 resets)
                e9 = cp.tile([128, FD9], f32, tag="e9")
                nc.vector.tensor_tensor_scan(
                    out=e9[:], data0=segpat[:], data1=f9[:], initial=0.0,
                    op0=Alu.mult, op1=Alu.max,
                )
                # ---- POOL: first-spike marker ----
                # e9 is a non-decreasing 0/1 flag per segment, so
                # e9[t-1] + e9[t] == 1  <=>  first spike at t.  A single
                # Pool TT here: a second Pool op would overlap the m9 scan
                # and SBUF-port contention slows that scan by ~60%.
                w = wp.tile([128, FD], f32, tag="w")
                nc.gpsimd.tensor_tensor(
                    out=r3(w[:]), in0=r3(e9[:], TBP)[:, :, 0:TB],
                    in1=r3(e9[:], TBP)[:, :, 1:TBP], op=Alu.add,
                )

                if not last:
                    # h9 = -8*e9 + any   (slot0: h=any, the v_init gate);
                    # 'any' is e9 slot 8, broadcast over the slots.  keep is
                    # h9 < 1  <=>  spiked-by-t OR no spike in the block.
                    h9 = hp.tile([128, FD9], f32, tag="h9")
                    nc.vector.scalar_tensor_tensor(
                        out=r3(h9[:], TBP), in0=r3(e9[:], TBP), scalar=-8.0,
                        in1=r3(e9[:], TBP)[:, :, 8:9].broadcast_to(
                            [128, B, TBP]),
                        op0=Alu.mult, op1=Alu.add,
                    )

                if not last:
                    # d29 = (h9 < 1) * vx9   (mask x AND gate v_init)
                    d29 = dp.tile([128, FD9], f32, tag="d29")
                    nc.vector.scalar_tensor_tensor(
                        out=d29[:], in0=h9[:], scalar=1.0, in1=vx_n[:],
                        op0=Alu.is_lt, op1=Alu.mult,
                    )
                    # m9 = membrane scan for block k+1
                    m9n = mp.tile([128, FD9], f32, tag="m9")
                    nc.vector.tensor_tensor_scan(
                        out=m9n[:], data0=bpat[:], data1=d29[:], initial=0.0,
                        op0=Alu.mult, op1=Alu.add,
                    )
                    # mh9 = (m9-1)*invg  (slots 1..8; slot0 stays -1)
                    nc.vector.scalar_tensor_tensor(
                        out=mh_sl, in0=r3(m9n[:], TBP)[:, :, 1:TBP],
                        scalar=1.0, in1=r3(invg[:]), op0=Alu.subtract,
                        op1=Alu.mult,
                    )

                # The tile scheduler enqueues per engine by sim-ready
                # time and its Pool cost model is optimistic; the wait floor
                # (sim-only) keeps spk/u/a' from jumping ahead of the m9
                # chain and stalling the real Vector queue on the Pool w.
                with tc.tile_wait_until((k + 2) * 0.02):
                    spk = spkp.tile([128, FD], f32, tag="spk")
                    nc.vector.scalar_tensor_tensor(
                        out=spk[:], in0=w[:], scalar=1.0, in1=qt[:],
                        op0=Alu.is_equal, op1=Alu.mult,
                    )
                    if not last:
                        # u = p^(6-tau*) if spiked else 0 (1-hot sum)
                        u = up.tile([128, B], f32, tag="u")
                        nc.vector.tensor_reduce(
                            out=u[:], in_=r3(spk[:]), axis=Ax.X, op=Alu.add,
                        )
                        # a' = a*p8 + u
                        nc.vector.scalar_tensor_tensor(
                            out=a_out[:], in0=a_in[:], scalar=p8_col,
                            in1=u[:], op0=Alu.mult, op1=Alu.add,
                        )

                # ---- input prefetch for k+2; slot0 seed for block k+2 ----
                if k + 2 < nblk:
                    vx_c, vx_n = vx_n, stage_x(k + 2)
                    nc.scalar.copy(
                        out=r3(vx_n[:], TBP)[:, :, 0:1],
                        in_=r3(m9n[:], TBP)[:, :, 8:9],
                    )
                else:
                    vx_c = vx_n

                # ---- output path: transpose spk, convert bf16, DMA ----
                spkT = psout.tile([128, FD], f32, tag="spkT")
                nc.tensor.transpose(spkT[:, 0:128], spk[:, 0:128], ident[:])
                nc.tensor.transpose(spkT[:, 128:256], spk[:, 128:256], ident[:])
                outb = outp.tile([128, FD], bf16, tag="outb")
                nc.scalar.copy(out=outb[:], in_=spkT[:])
                nc.scalar.dma_start(out=dram_block_ap(y_d[:], k, 2), in_=outb[:])

                if not last:
                    m9 = m9n
                    a_in, a_out = a_out, a_in

    nc.compile()
    return nc


def _host_consts(beta_raw, p_raw, b_raw, core):
    sh = slice(core * NSH, (core + 1) * NSH)
    beta = np.clip(beta_raw[sh], 0.001, 0.999).astype(np.float32)
    p = np.clip(np.abs(p_raw[sh]), 0.0, 0.999).astype(np.float32)
    bb = np.clip(np.abs(b_raw[sh]), 0.001, 1.0).astype(np.float32)

    tau = np.arange(TB, dtype=np.float32)
    bpat = np.tile(
        np.concatenate([[0.0] * 1, [1.0] * TB]).astype(np.float32)[None, :],
        (NSH, B),
    ) * beta[:, None]  # slot0 -> 0, others beta[n]
    segpat = np.tile(
        np.concatenate([[0.0], np.ones(TB)]).astype(np.float32)[None, :],
        (NSH, B),
    )
    q = (p[:, None] ** (6.0 - tau[None, :])).astype(np.float32)  # [NSH, TB]
    qt = np.tile(q, (1, B))
    ppow = (p[:, None] ** (tau[None, :] + 1.0)).astype(np.float32)
    invg = np.tile((1.0 / (bb[:, None] * ppow)).astype(np.float32), (1, B))
    prm = (p ** 8).astype(np.float32)[:, None]
    return {
        "bpat": np.ascontiguousarray(bpat),
        "segpat": np.ascontiguousarray(segpat),
        "qt": np.ascontiguousarray(qt),
        "invg": np.ascontiguousarray(invg),
        "prm": np.ascontiguousarray(prm),
    }


def build_in_maps(x, beta_raw, p_raw, b_raw, t_len=T_LEN):
    in_maps = []
    for core in range(NCORES):
        sh = slice(core * NSH, (core + 1) * NSH)
        m = {"x": np.ascontiguousarray(x[:t_len, :, sh], dtype=np.float32)}
        m.update(_host_consts(beta_raw, p_raw, b_raw, core))
        in_maps.append(m)
    return in_maps


def get_module(t_len=T_LEN, repeats=1, rep_reset=True, variant="full"):
    key = (t_len, repeats, rep_reset, variant)
    if key not in _MODULE_CACHE:
        _MODULE_CACHE[key] = _build_module(t_len, repeats, rep_reset, variant)
    return _MODULE_CACHE[key]


def kernel(x, beta_raw, p_raw, b_raw):
    _import_concourse()
    from concourse.bass_utils import run_bass_kernel_spmd

    nc = get_module(T_LEN)
    in_maps = build_in_maps(x, beta_raw, p_raw, b_raw)
    res = run_bass_kernel_spmd(nc, in_maps, core_ids=list(range(NCORES)))
    y = np.concatenate([res.results[c]["y"] for c in range(NCORES)], axis=2)
    # spk*q values in bf16; threshold to {0,1} spikes on the host
    return (y.astype(np.float32) > 0.0).astype(np.float32)


if __name__ == "__main__":
    xs = np.random.RandomState(0).randn(T_LEN, B, N).astype(np.float32) * 0.6
    br = np.random.RandomState(1).uniform(0.7, 0.99, N).astype(np.float32)
    pr = np.random.RandomState(2).uniform(0.5, 0.95, N).astype(np.float32)
    brw = np.random.RandomState(3).uniform(0.2, 1.0, N).astype(np.float32)
    out = kernel(xs, br, pr, brw)
    print(out.shape, out.dtype, out.mean())
